# revision 26
# baseline (speedup 1.0000x reference)
"""Trainium2 Bass kernel for nn_Attention_28552942584284 (sparse_attention).

Reference computation (per batch b):
    scores  = exp(Q @ K^T) / sqrt(d)          # exp BEFORE scaling
    scores  = where(mask == 0, -1e9, scores)
    p_attn  = softmax(scores, axis=-1)
    out     = (p_attn @ V).sum(axis=q)        # == colsum(p_attn) @ V
    returns (out, p_attn)

Sharding: data-parallel over batch B=16 across 8 cores (2 batches/core).

Per-core device pipeline (scores stay in [q-partition, kv-free] layout):
    qk       = Qt.T @ Kt                      (bf16 matmul, PSUM fp32)
    e1       = Exp(qk)                        (ACT, PSUM->SBUF fp32)
    t        = mask*2048 + e1                 (DVE scalar_tensor_tensor, u8 mask)
    p_un, Z  = Exp(t/sqrt(d) - 128)           (ACT, accum_out => rowsum;
                                               masked lanes underflow to 0.0,
                                               matching reference exactly)
    recip    = 1/Z                            (DVE reciprocal)
    p_attn   = p_un * recip                   (DVE tensor_scalar, -> DRAM)
    c        = colsum(p_attn) = recip.T @ p_un  (PE matmul, float32r)
    out      = c.T @ V                        (PE matmul fp32 after tiny PE
                                               transposes of c)

The softmax max-subtraction is skipped deliberately: unmasked scores lie in
~[0.05, 0.08] and masked scores are -1e9, so exp() is numerically safe and
exp(-1e9) == 0.0 in fp32 exactly as in the reference.
"""

import math

import numpy as np
import ml_dtypes

import concourse.bass as bass
import concourse.bacc as bacc
import concourse.mybir as mybir
import concourse.tile as tile
from concourse.bass import ts
from concourse.bass_utils import run_bass_kernel_spmd

# Problem constants (hardcoded; kernel.py must be self-contained).
B, SQ, SKV, D = 16, 1024, 1024, 256
NCORES = 8
NB = B // NCORES  # batches per core
P = 128
# Mask handling: the device computes t = mask*LAM + e1 (scalar_tensor_tensor
# on the uint8 mask), then p_un = Exp(t/sqrt(d) - LAM/sqrt(d)).  Unmasked
# (mask=1): exp(e1/16 + (LAM-LAM)/16) = exp(e1/16).  Masked (mask=0):
# exp(e1/16 - 128) < 1e-55 -> exactly 0.0 in fp32, matching the reference's
# exp(-1e9 - max).  LAM=2048 keeps e1's low bits: ulp(2048+e1) = 2^-12*2^11
# => p_attn rel err ~1.5e-5.
LAM = 2048.0
INV_SQRT_D = 1.0 / math.sqrt(D)

F32 = mybir.dt.float32
F32R = mybir.dt.float32r
BF16 = mybir.dt.bfloat16
U8 = mybir.dt.uint8
EXP = mybir.ActivationFunctionType.Exp

# Tunables
# Colsum matmul dtype strategy:
#   "f32r_copy"   - extra DVE copies round (recip, p_un) to float32r (e8m11)
#                   so the colsum matmul streams at full rate; p_attn stays
#                   full fp32 precision.
#   "f32r_direct" - exp2 writes p_un as float32r directly (saves the copy,
#                   p_attn inherits ~2.4e-4 rounding).
#   "f32"         - plain fp32 matmul (4 cycles/row on the PE).
COLSUM_MODE = "f32r_copy"
MQT = 8  # q-tiles of mask loaded per DMA
PQT = 2  # q-tiles of p_attn stored per DMA


def build(nb=NB, sq=SQ, skv=SKV, d=D, repeat=1):
    """Build the single-core Bass program (SPMD across cores).

    repeat > 1 re-runs the whole body (same I/O) for benchmarking: the time
    difference between repeat=R and repeat=1 programs isolates steady-state
    kernel time from fixed NEFF/dispatch overheads."""
    assert d % P == 0 and sq % P == 0 and skv % 512 == 0
    nd = d // P  # contraction chunks (2)
    nqt = sq // P  # q tiles (8)
    nkv = skv // P  # kv 128-chunks (8)
    nvc = skv // 512  # kv 512-chunks (2)
    mqt = min(MQT, nqt)
    pqt = min(PQT, nqt)

    nc = bacc.Bacc("TRN2", target_bir_lowering=False, debug=False)
    qt_d = nc.dram_tensor("qt", [nb, d, sq], BF16, kind="ExternalInput")
    kt_d = nc.dram_tensor("kt", [nb, d, skv], BF16, kind="ExternalInput")
    v_d = nc.dram_tensor("v", [nb, skv, d], F32, kind="ExternalInput")
    m_d = nc.dram_tensor("m", [nb, sq, skv], U8, kind="ExternalInput")
    pa_d = nc.dram_tensor("pa", [nb, sq, skv], F32, kind="ExternalOutput")
    o1_d = nc.dram_tensor("o1", [nb, d], F32, kind="ExternalOutput")

    with tile.TileContext(nc) as tc:
        with (
            tc.tile_pool(name="qk_pool", bufs=2) as qk_pool,
            tc.tile_pool(name="v_pool", bufs=2) as v_pool,
            tc.tile_pool(name="m_pool", bufs=2) as m_pool,
            tc.tile_pool(name="e1_pool", bufs=3) as e1_pool,
            tc.tile_pool(name="t_pool", bufs=3) as t_pool,
            tc.tile_pool(name="pu_pool", bufs=3) as pu_pool,
            tc.tile_pool(name="pa_pool", bufs=3) as pa_pool,
            tc.tile_pool(name="small_pool", bufs=8) as small_pool,
            tc.tile_pool(name="c_pool", bufs=2) as c_pool,
            tc.tile_pool(name="ps_pool", bufs=4, space=bass.MemorySpace.PSUM) as ps_pool,
            tc.tile_pool(name="cs_pool", bufs=1, space=bass.MemorySpace.PSUM) as cs_pool,
            tc.tile_pool(name="f_pool", bufs=1, space=bass.MemorySpace.PSUM) as f_pool,
            tc.tile_pool(name="tp_pool", bufs=1, space=bass.MemorySpace.PSUM) as tp_pool,
        ):
            ones1 = c_pool.tile([1, 1], F32, tag="ones1")
            nc.vector.memset(ones1, 1.0)
            exp2_bias = c_pool.tile([P, 1], F32, tag="exp2_bias")
            nc.vector.memset(exp2_bias, -float(LAM * INV_SQRT_D))
            for b in [b for _ in range(repeat) for b in range(nb)]:
                # Q^T / K^T: one [128, nd, seq] bf16 tile each (partition p of
                # d-chunk dc holds row dc*128+p), one contiguous DMA each.
                qt_sb = qk_pool.tile([P, nd, sq], BF16, tag="qt_sb")
                nc.sync.dma_start(
                    out=qt_sb, in_=qt_d[b].rearrange("(c p) q -> p c q", p=P)
                )
                kt_sb = qk_pool.tile([P, nd, skv], BF16, tag="kt_sb")
                nc.sync.dma_start(
                    out=kt_sb, in_=kt_d[b].rearrange("(c p) q -> p c q", p=P)
                )
                qts = [qt_sb[:, dc, :] for dc in range(nd)]
                kts = [kt_sb[:, dc, :] for dc in range(nd)]
                # colsum accumulator: c[0, kv] += recip.T @ p_unnorm
                cs = cs_pool.tile([1, skv], F32)

                for i in range(nqt):
                    # mask: merged load of MQT q-tiles per DMA, uint8.
                    if i % mqt == 0:
                        mt = m_pool.tile([P, mqt, skv], U8, tag="mt")
                        i4 = i // mqt
                        nc.sync.dma_start(
                            out=mt,
                            in_=m_d[b, ts(i4, mqt * P), :].rearrange(
                                "(t p) k -> p t k", p=P
                            ),
                        )

                    e1 = e1_pool.tile([P, skv], F32, tag="e1")
                    for n in range(nvc):
                        ps = ps_pool.tile([P, 512], F32, tag="ps")
                        for dc in range(nd):
                            nc.tensor.matmul(
                                ps,
                                lhsT=qts[dc][:, ts(i, P)],
                                rhs=kts[dc][:, ts(n, 512)],
                                start=(dc == 0),
                                stop=(dc == nd - 1),
                            )
                        # e1 = exp(qk); scores scaling by 1/sqrt(d) is folded
                        # into the second exp below.
                        nc.scalar.activation(e1[:, ts(n, 512)], ps, EXP)

                    # t = mask*LAM + e1
                    t = t_pool.tile([P, skv], F32, tag="t")
                    nc.vector.scalar_tensor_tensor(
                        t,
                        in0=mt[:, i % mqt, :],
                        scalar=float(LAM),
                        in1=e1,
                        op0=mybir.AluOpType.mult,
                        op1=mybir.AluOpType.add,
                    )

                    pu_dt = F32R if COLSUM_MODE == "f32r_direct" else F32
                    pu = pu_pool.tile([P, skv], pu_dt, tag="pu")
                    z = small_pool.tile([P, 1], F32, tag="z")
                    nc.scalar.activation(
                        pu,
                        t,
                        EXP,
                        scale=float(INV_SQRT_D),
                        bias=exp2_bias[:, :],
                        accum_out=z,
                    )
                    rc = small_pool.tile([P, 1], F32, tag="rc")
                    nc.vector.reciprocal(rc, z)

                    # p_attn tiles are batched PQT q-tiles per store DMA.
                    if i % pqt == 0:
                        pat = pa_pool.tile([P, pqt, skv], F32, tag="pat")
                    pu_f32 = pu.bitcast(F32) if COLSUM_MODE == "f32r_direct" else pu
                    nc.vector.tensor_scalar_mul(pat[:, i % pqt, :], pu_f32, rc)
                    if i % pqt == pqt - 1:
                        i2 = i // pqt
                        nc.sync.dma_start(
                            out=pa_d[b, ts(i2, pqt * P), :].rearrange(
                                "(t p) k -> p t k", p=P
                            ),
                            in_=pat,
                        )

                    # colsum of p_attn: out[1, kv] += sum_q recip[q]*pu[q, kv]
                    if COLSUM_MODE == "f32":
                        rc_mm, pu_mm = rc, pu
                    elif COLSUM_MODE == "f32r_direct":
                        rc_mm = small_pool.tile([P, 1], F32R, tag="rc_r")
                        nc.vector.tensor_copy(rc_mm, rc)
                        pu_mm = pu
                    else:  # f32r_copy
                        rc_mm = small_pool.tile([P, 1], F32R, tag="rc_r")
                        nc.vector.tensor_copy(rc_mm, rc)
                        pu_mm = pu_pool.tile([P, skv], F32R, tag="pu_r")
                        nc.vector.tensor_copy(pu_mm, pu)
                    for n in range(nvc):
                        nc.tensor.matmul(
                            cs[:, ts(n, 512)],
                            lhsT=rc_mm,
                            rhs=pu_mm[:, ts(n, 512)],
                            start=(i == 0),
                            stop=(i == nqt - 1),
                        )

                # V: one [128, nkv, d] fp32 tile (partition p holds kv rows
                # {j*128+p}), one DMA. Emitted after the q-loop so it doesn't
                # delay the first mask loads in the SP DMA FIFO; only the
                # final matmuls consume it.
                v_sb = v_pool.tile([P, nkv, d], F32, tag="v_sb")
                nc.sync.dma_start(
                    out=v_sb, in_=v_d[b].rearrange("(j p) d -> p j d", p=P)
                )

                # Evacuate colsum, then transpose [1, skv] -> [128, nkv]
                # (ct[p, j] = c[j*128 + p]) via 8 tiny PE transposes.
                c_sb = c_pool.tile([1, skv], F32, tag="c_sb")
                nc.vector.tensor_copy(c_sb, cs)
                ct = c_pool.tile([P, nkv], F32, tag="ct")
                for j in range(nkv):
                    tp = tp_pool.tile([P, 1], F32, tag="tp")
                    nc.tensor.transpose(tp, c_sb[0:1, ts(j, P)], ones1)
                    nc.vector.tensor_copy(ct[:, ts(j, 1)], tp)

                # Final out = c.T @ V: small (8 matmuls of N=256), plain fp32.
                f_ps = f_pool.tile([1, d], F32)
                for j in range(nkv):
                    nc.tensor.matmul(
                        f_ps,
                        lhsT=ct[:, ts(j, 1)],
                        rhs=v_sb[:, j, :],
                        start=(j == 0),
                        stop=(j == nkv - 1),
                    )
                o_sb = c_pool.tile([1, d], F32, tag="o_sb")
                nc.scalar.copy(o_sb, f_ps)
                nc.sync.dma_start(out=o1_d[ts(b, 1), :], in_=o_sb)

    nc.compile()
    return nc


_CACHE = {}


def _built():
    if "nc" not in _CACHE:
        _CACHE["nc"] = build()
    return _CACHE["nc"]


def _prep_inputs(query, key, value, mask):
    """Host-side prep: cast/transpose Q,K to bf16 [B, D, S]; mask -> uint8;
    shard over batch."""
    qT = np.ascontiguousarray(query.transpose(0, 2, 1)).astype(ml_dtypes.bfloat16)
    kT = np.ascontiguousarray(key.transpose(0, 2, 1)).astype(ml_dtypes.bfloat16)
    v = np.ascontiguousarray(value.astype(np.float32))
    m8 = (np.asarray(mask) != 0).astype(np.uint8)
    in_maps = []
    for c in range(NCORES):
        s = slice(c * NB, (c + 1) * NB)
        in_maps.append(
            {
                "qt": np.ascontiguousarray(qT[s]),
                "kt": np.ascontiguousarray(kT[s]),
                "v": v[s],
                "m": np.ascontiguousarray(m8[s]),
            }
        )
    return in_maps


def run(query, key, value, mask, **spmd_kwargs):
    """Run on 8 NeuronCores; returns (results, BassKernelResults)."""
    query = np.asarray(query, dtype=np.float32)
    key = np.asarray(key, dtype=np.float32)
    value = np.asarray(value, dtype=np.float32)
    mask = np.asarray(mask)
    nc = _built()
    in_maps = _prep_inputs(query, key, value, mask)
    res = run_bass_kernel_spmd(nc, in_maps, core_ids=list(range(NCORES)), **spmd_kwargs)
    out1 = np.concatenate([r["o1"] for r in res.results], axis=0).astype(np.float32)
    p_attn = np.concatenate([r["pa"] for r in res.results], axis=0).astype(np.float32)
    return (out1, p_attn), res


def kernel(query, key, value, mask):
    outs, _ = run(query, key, value, mask)
    return outs


# revision 27
# speedup vs baseline: 2.1615x; 2.1615x over previous
"""Trainium2 Bass kernel for nn_Attention_28552942584284 (sparse_attention).

Reference computation (per batch b):
    scores  = exp(Q @ K^T) / sqrt(d)          # exp BEFORE scaling
    scores  = where(mask == 0, -1e9, scores)
    p_attn  = softmax(scores, axis=-1)
    out     = (p_attn @ V).sum(axis=q)        # == colsum(p_attn) @ V
    returns (out, p_attn)

Sharding: data-parallel over batch B=16 across 8 cores (2 batches/core).

Per-core device pipeline (scores stay in [q-partition, kv-free] layout):
    qk       = Qt.T @ Kt                      (bf16 matmul, PSUM fp32)
    e1       = Exp(qk)                        (ACT, PSUM->SBUF fp32)
    t        = mask*2048 + e1                 (DVE scalar_tensor_tensor, u8 mask)
    p_un, Z  = Exp(t/sqrt(d) - 128)           (ACT, accum_out => rowsum;
                                               masked lanes underflow to 0.0,
                                               matching reference exactly)
    recip    = 1/Z                            (DVE reciprocal)
    p_attn   = p_un * recip                   (DVE tensor_scalar, -> DRAM)
    c        = colsum(p_attn) = recip.T @ p_un  (PE matmul, float32r)
    out      = c.T @ V                        (PE matmul fp32 after tiny PE
                                               transposes of c)

The softmax max-subtraction is skipped deliberately: unmasked scores lie in
~[0.05, 0.08] and masked scores are -1e9, so exp() is numerically safe and
exp(-1e9) == 0.0 in fp32 exactly as in the reference.

Host-side prep (cheap, done in kernel()): Q,K transposed to [B, D, S] and
cast to bf16 (error on p_attn ~1e-5: the double-exp flattens qk rounding);
mask shipped as uint8.  Per-core HBM traffic is 13.5 MB (qt 0.5 + kt 0.5 +
v 2.0 + mask 2.1 + p_attn out 8.4), which at ~360-420 GB/s/core makes the
kernel DMA-bound: measured steady-state ~32-40 us/kernel on HW (DMA-only
floor for the same traffic measured ~34 us; CoreSim cost model: ~57 us with
SP/DMA the top engine at 45 us busy, ACT 41, DVE 39, PE 29).  Errors vs the
fp32 jax reference: p_attn rel2 ~7e-6 (absmax/scale 3.6e-5), out rel2 ~7e-6
(absmax/scale 4.7e-6; elementwise relmax on near-zero elements ~0.4 comes
from float32r (e8m11) rounding noise of ~1.3e-3 absolute against out scale
~291 -- see COLSUM_MODE to trade PE time for exactness).
"""

import math

import numpy as np
import ml_dtypes

import concourse.bass as bass
import concourse.bacc as bacc
import concourse.mybir as mybir
import concourse.tile as tile
from concourse.bass import ts
from concourse.bass_utils import run_bass_kernel_spmd

# Problem constants (hardcoded; kernel.py must be self-contained).
B, SQ, SKV, D = 16, 1024, 1024, 256
NCORES = 8
NB = B // NCORES  # batches per core
P = 128
# Mask handling: the device computes t = mask*LAM + e1 (scalar_tensor_tensor
# on the uint8 mask), then p_un = Exp(t/sqrt(d) - LAM/sqrt(d)).  Unmasked
# (mask=1): exp(e1/16 + (LAM-LAM)/16) = exp(e1/16).  Masked (mask=0):
# exp(e1/16 - 128) < 1e-55 -> exactly 0.0 in fp32, matching the reference's
# exp(-1e9 - max).  LAM=2048 keeps e1's low bits: ulp(2048+e1) = 2^-12*2^11
# => p_attn rel err ~1.5e-5.
LAM = 2048.0
INV_SQRT_D = 1.0 / math.sqrt(D)

F32 = mybir.dt.float32
F32R = mybir.dt.float32r
BF16 = mybir.dt.bfloat16
U8 = mybir.dt.uint8
EXP = mybir.ActivationFunctionType.Exp

# Tunables
# Colsum matmul dtype strategy:
#   "f32r_copy"   - extra DVE copies round (recip, p_un) to float32r (e8m11)
#                   so the colsum matmul streams at full rate; p_attn stays
#                   full fp32 precision.
#   "f32r_direct" - exp2 writes p_un as float32r directly (saves the copy,
#                   p_attn inherits ~2.4e-4 rounding).
#   "f32"         - plain fp32 matmul (4 cycles/row on the PE).
COLSUM_MODE = "f32r_copy"
MQT = 8  # q-tiles of mask loaded per DMA
PQT = 2  # q-tiles of p_attn stored per DMA


def build(nb=NB, sq=SQ, skv=SKV, d=D, repeat=1):
    """Build the single-core Bass program (SPMD across cores).

    repeat > 1 re-runs the whole body (same I/O) for benchmarking: the time
    difference between repeat=R and repeat=1 programs isolates steady-state
    kernel time from fixed NEFF/dispatch overheads."""
    assert d % P == 0 and sq % P == 0 and skv % 512 == 0
    nd = d // P  # contraction chunks (2)
    nqt = sq // P  # q tiles (8)
    nkv = skv // P  # kv 128-chunks (8)
    nvc = skv // 512  # kv 512-chunks (2)
    mqt = min(MQT, nqt)
    pqt = min(PQT, nqt)

    nc = bacc.Bacc("TRN2", target_bir_lowering=False, debug=False)
    qt_d = nc.dram_tensor("qt", [nb, d, sq], BF16, kind="ExternalInput")
    kt_d = nc.dram_tensor("kt", [nb, d, skv], BF16, kind="ExternalInput")
    v_d = nc.dram_tensor("v", [nb, skv, d], F32, kind="ExternalInput")
    m_d = nc.dram_tensor("m", [nb, sq, skv], U8, kind="ExternalInput")
    pa_d = nc.dram_tensor("pa", [nb, sq, skv], F32, kind="ExternalOutput")
    o1_d = nc.dram_tensor("o1", [nb, d], F32, kind="ExternalOutput")

    with tile.TileContext(nc) as tc:
        with (
            tc.tile_pool(name="qk_pool", bufs=2) as qk_pool,
            tc.tile_pool(name="v_pool", bufs=2) as v_pool,
            tc.tile_pool(name="m_pool", bufs=2) as m_pool,
            tc.tile_pool(name="e1_pool", bufs=3) as e1_pool,
            tc.tile_pool(name="t_pool", bufs=3) as t_pool,
            tc.tile_pool(name="pu_pool", bufs=3) as pu_pool,
            tc.tile_pool(name="pa_pool", bufs=3) as pa_pool,
            tc.tile_pool(name="small_pool", bufs=8) as small_pool,
            tc.tile_pool(name="c_pool", bufs=2) as c_pool,
            tc.tile_pool(name="ps_pool", bufs=4, space=bass.MemorySpace.PSUM) as ps_pool,
            tc.tile_pool(name="cs_pool", bufs=1, space=bass.MemorySpace.PSUM) as cs_pool,
            tc.tile_pool(name="f_pool", bufs=1, space=bass.MemorySpace.PSUM) as f_pool,
            tc.tile_pool(name="tp_pool", bufs=1, space=bass.MemorySpace.PSUM) as tp_pool,
        ):
            ones1 = c_pool.tile([1, 1], F32, tag="ones1")
            nc.vector.memset(ones1, 1.0)
            exp2_bias = c_pool.tile([P, 1], F32, tag="exp2_bias")
            nc.vector.memset(exp2_bias, -float(LAM * INV_SQRT_D))
            for b in [b for _ in range(repeat) for b in range(nb)]:
                # Q^T / K^T: one [128, nd, seq] bf16 tile each (partition p of
                # d-chunk dc holds row dc*128+p), one contiguous DMA each.
                qt_sb = qk_pool.tile([P, nd, sq], BF16, tag="qt_sb")
                nc.sync.dma_start(
                    out=qt_sb, in_=qt_d[b].rearrange("(c p) q -> p c q", p=P)
                )
                kt_sb = qk_pool.tile([P, nd, skv], BF16, tag="kt_sb")
                nc.sync.dma_start(
                    out=kt_sb, in_=kt_d[b].rearrange("(c p) q -> p c q", p=P)
                )
                qts = [qt_sb[:, dc, :] for dc in range(nd)]
                kts = [kt_sb[:, dc, :] for dc in range(nd)]
                # colsum accumulator: c[0, kv] += recip.T @ p_unnorm
                cs = cs_pool.tile([1, skv], F32)

                for i in range(nqt):
                    # mask: merged load of MQT q-tiles per DMA, uint8.
                    if i % mqt == 0:
                        mt = m_pool.tile([P, mqt, skv], U8, tag="mt")
                        i4 = i // mqt
                        nc.sync.dma_start(
                            out=mt,
                            in_=m_d[b, ts(i4, mqt * P), :].rearrange(
                                "(t p) k -> p t k", p=P
                            ),
                        )

                    e1 = e1_pool.tile([P, skv], F32, tag="e1")
                    for n in range(nvc):
                        ps = ps_pool.tile([P, 512], F32, tag="ps")
                        for dc in range(nd):
                            nc.tensor.matmul(
                                ps,
                                lhsT=qts[dc][:, ts(i, P)],
                                rhs=kts[dc][:, ts(n, 512)],
                                start=(dc == 0),
                                stop=(dc == nd - 1),
                            )
                        # e1 = exp(qk); scores scaling by 1/sqrt(d) is folded
                        # into the second exp below.
                        nc.scalar.activation(e1[:, ts(n, 512)], ps, EXP)

                    # t = mask*LAM + e1
                    t = t_pool.tile([P, skv], F32, tag="t")
                    nc.vector.scalar_tensor_tensor(
                        t,
                        in0=mt[:, i % mqt, :],
                        scalar=float(LAM),
                        in1=e1,
                        op0=mybir.AluOpType.mult,
                        op1=mybir.AluOpType.add,
                    )

                    pu_dt = F32R if COLSUM_MODE == "f32r_direct" else F32
                    pu = pu_pool.tile([P, skv], pu_dt, tag="pu")
                    z = small_pool.tile([P, 1], F32, tag="z")
                    nc.scalar.activation(
                        pu,
                        t,
                        EXP,
                        scale=float(INV_SQRT_D),
                        bias=exp2_bias[:, :],
                        accum_out=z,
                    )
                    rc = small_pool.tile([P, 1], F32, tag="rc")
                    nc.vector.reciprocal(rc, z)

                    # p_attn tiles are batched PQT q-tiles per store DMA.
                    if i % pqt == 0:
                        pat = pa_pool.tile([P, pqt, skv], F32, tag="pat")
                    pu_f32 = pu.bitcast(F32) if COLSUM_MODE == "f32r_direct" else pu
                    nc.vector.tensor_scalar_mul(pat[:, i % pqt, :], pu_f32, rc)
                    if i % pqt == pqt - 1:
                        i2 = i // pqt
                        nc.sync.dma_start(
                            out=pa_d[b, ts(i2, pqt * P), :].rearrange(
                                "(t p) k -> p t k", p=P
                            ),
                            in_=pat,
                        )

                    # colsum of p_attn: out[1, kv] += sum_q recip[q]*pu[q, kv]
                    if COLSUM_MODE == "f32":
                        rc_mm, pu_mm = rc, pu
                    elif COLSUM_MODE == "f32r_direct":
                        rc_mm = small_pool.tile([P, 1], F32R, tag="rc_r")
                        nc.vector.tensor_copy(rc_mm, rc)
                        pu_mm = pu
                    else:  # f32r_copy
                        rc_mm = small_pool.tile([P, 1], F32R, tag="rc_r")
                        nc.vector.tensor_copy(rc_mm, rc)
                        pu_mm = pu_pool.tile([P, skv], F32R, tag="pu_r")
                        nc.vector.tensor_copy(pu_mm, pu)
                    for n in range(nvc):
                        nc.tensor.matmul(
                            cs[:, ts(n, 512)],
                            lhsT=rc_mm,
                            rhs=pu_mm[:, ts(n, 512)],
                            start=(i == 0),
                            stop=(i == nqt - 1),
                        )

                # V: one [128, nkv, d] fp32 tile (partition p holds kv rows
                # {j*128+p}), one DMA. Emitted after the q-loop so it doesn't
                # delay the first mask loads in the SP DMA FIFO; only the
                # final matmuls consume it.
                v_sb = v_pool.tile([P, nkv, d], F32, tag="v_sb")
                nc.sync.dma_start(
                    out=v_sb, in_=v_d[b].rearrange("(j p) d -> p j d", p=P)
                )

                # Evacuate colsum, then transpose [1, skv] -> [128, nkv]
                # (ct[p, j] = c[j*128 + p]) via 8 tiny PE transposes.
                c_sb = c_pool.tile([1, skv], F32, tag="c_sb")
                nc.vector.tensor_copy(c_sb, cs)
                ct = c_pool.tile([P, nkv], F32, tag="ct")
                for j in range(nkv):
                    tp = tp_pool.tile([P, 1], F32, tag="tp")
                    nc.tensor.transpose(tp, c_sb[0:1, ts(j, P)], ones1)
                    nc.vector.tensor_copy(ct[:, ts(j, 1)], tp)

                # Final out = c.T @ V: small (8 matmuls of N=256), plain fp32.
                f_ps = f_pool.tile([1, d], F32)
                for j in range(nkv):
                    nc.tensor.matmul(
                        f_ps,
                        lhsT=ct[:, ts(j, 1)],
                        rhs=v_sb[:, j, :],
                        start=(j == 0),
                        stop=(j == nkv - 1),
                    )
                o_sb = c_pool.tile([1, d], F32, tag="o_sb")
                nc.scalar.copy(o_sb, f_ps)
                nc.sync.dma_start(out=o1_d[ts(b, 1), :], in_=o_sb)

    nc.compile()
    return nc


_CACHE = {}


def _built():
    if "nc" not in _CACHE:
        _CACHE["nc"] = build()
    return _CACHE["nc"]


def _prep_inputs(query, key, value, mask):
    """Host-side prep: cast/transpose Q,K to bf16 [B, D, S]; mask -> uint8;
    shard over batch."""
    qT = np.ascontiguousarray(query.transpose(0, 2, 1)).astype(ml_dtypes.bfloat16)
    kT = np.ascontiguousarray(key.transpose(0, 2, 1)).astype(ml_dtypes.bfloat16)
    v = np.ascontiguousarray(value.astype(np.float32))
    m8 = (np.asarray(mask) != 0).astype(np.uint8)
    in_maps = []
    for c in range(NCORES):
        s = slice(c * NB, (c + 1) * NB)
        in_maps.append(
            {
                "qt": np.ascontiguousarray(qT[s]),
                "kt": np.ascontiguousarray(kT[s]),
                "v": v[s],
                "m": np.ascontiguousarray(m8[s]),
            }
        )
    return in_maps


def run(query, key, value, mask, **spmd_kwargs):
    """Run on 8 NeuronCores; returns (results, BassKernelResults)."""
    query = np.asarray(query, dtype=np.float32)
    key = np.asarray(key, dtype=np.float32)
    value = np.asarray(value, dtype=np.float32)
    mask = np.asarray(mask)
    nc = _built()
    in_maps = _prep_inputs(query, key, value, mask)
    res = run_bass_kernel_spmd(nc, in_maps, core_ids=list(range(NCORES)), **spmd_kwargs)
    out1 = np.concatenate([r["o1"] for r in res.results], axis=0).astype(np.float32)
    p_attn = np.concatenate([r["pa"] for r in res.results], axis=0).astype(np.float32)
    return (out1, p_attn), res


def kernel(query, key, value, mask):
    outs, _ = run(query, key, value, mask)
    return outs
